# revision 9
# baseline (speedup 1.0000x reference)
"""Trainium2 Bass kernel: 16-head self-attention (B=4, S=2048, E=1024).

Reference math:
  Q = x @ W_q.T ; K = x @ W_k.T ; V = x @ W_v.T      (split into 16 heads of 64)
  A = softmax(Q K^T / sqrt(64)) ; Hout = A @ V
  out = concat_heads(Hout) @ W_o.T + b_o

Sharding: data-parallel over (batch, seq-half) -> 8 cores, no collectives.
Core i handles batch i//2 and query rows [ (i%2)*1024, (i%2+1)*1024 ).
K/V are computed for the full 2048-token sequence on every core.  Odd cores
receive x^T with the two sequence halves swapped so the "first 1024 columns"
are always the core's queries; softmax is permutation-invariant over keys.

Projections and QK^T are bf16 (fp8 projections fail the 2e-2 error budget:
the uniform +-1/32 weights quantize badly in e4m3; QK's contraction is only
64, where DoubleRow measured 2x SLOWER).  Only the PV matmul uses fp8
e4m3 via MatmulPerfMode.DoubleRow, which contracts TWO 128-partition
k-tiles per instruction at the bf16 streaming rate (216ns per [*,2,512]
MM = 2x bf16 FLOPs).  Measured end-to-end rel err 1.67e-2 (gate 2e-2),
matching a numpy simulation of the same quantization points exactly.

On-chip layout (feature-on-partition, "transposed"):
  xT  [e, t]  bf16 (host pre-transposed)
  WqT/WkT/WvT/WoT [e, o] bf16; WvT host-scaled x16 (keeps fp8 V out of
      e4m3 denormals; compensated by a 16.0 ones-column)
  Q^T [o, t_q], K^T [o, t_k] bf16
  V stored fp8, kt-paired, slab padded to 80B (dual-fp8 ldweights needs
      even, 16B-aligned outer free-AP steps):
     VA8[k, ktp, h, j, 0:64] = 16*V(token (2*ktp+j)*128+k, head h),
     VA8[..., 64] = 16
  S^T[k, q] = matmul(lhsT=K^T_head, rhs=Q^T_head)   bf16, contraction d=64
  P = exp(S^T / 8) -> fp8 e4m3 directly from the ACT engine (no max
      subtraction needed: scores are N(0, ~0.33^2); the denominator uses
      the same quantized P so softmax weights still sum to 1)
  O^T[d,q] + denom row = DoubleRow matmul(lhsT=VA8 pair [128,2,65], rhs=P)
  Hout^T = O^T * (1/denom)
  Y[t, u] = matmul(lhsT=Hout^T tile, rhsT=WoT bf16) + b_o

Scheduling (trace-derived model): the binding constraint is the ACT
engine's exp stream plus PSUM-fabric contention between ACT psum reads
and PE psum writes — each [128,2,512] k-group costs ~3.1us regardless of
how much PE work overlaps it (a pure QK stream runs 186ns/MM; the same
stream next to ACT exp runs ~512ns/MM).  So all projection/output
matmuls are interleaved into the attention loop as filler where they
execute essentially for free, and PV is software-pipelined one k-group
behind exp so the PE never head-of-line blocks on ACT:
  phase A: K^T(kb0-2), V(heads 0-7), Q^T(chunk0,qb0)   -- dense matmuls
  phase B: attention(qb0) + K/Q/V-remainder fillers
  phase C: attention(qb1) + Q^T(qb1) fillers
  phase D: output projection (all rows)
"""

import sys

for _p in ("/opt/trn_rl_repo",):
    if _p not in sys.path:
        sys.path.append(_p)

import numpy as np
import ml_dtypes

import concourse.bass as bass
import concourse.mybir as mybir
import concourse.tile as tile
from concourse import bacc
from concourse.bass_utils import run_bass_kernel_spmd

B, S, E = 4, 2048, 1024
H, D = 16, 64
P = 128
SQ = S // 2  # queries per core
NCORES = 8
EC = E // P  # 8 feature chunks
ECP = EC // 2  # 4 chunk pairs (DoubleRow)
KT_TILES = S // P  # 16 key tiles
KTP = KT_TILES // 2  # 8 key-tile pairs
QB = 512  # q block (matmul free dim / PSUM bank width)
KG = 2  # k-tiles per exp group (ACT instruction spans KG*512 psum cols)
NQB = SQ // QB  # 2 q-blocks per core

BF16 = mybir.dt.bfloat16
FP8 = mybir.dt.float8e4
F32 = mybir.dt.float32
EXP = mybir.ActivationFunctionType.Exp
DR = mybir.MatmulPerfMode.DoubleRow

_CACHE = {}


def _dma_chunked(nc, dst, src_2d):
    """DMA a [E, N] DRAM tensor into SBUF [P, EC, N], one chunk at a time so
    consumers of chunk 0 don't wait for the whole transfer."""
    r = src_2d.rearrange("(c p) t -> p c t", p=P)
    for c in range(EC):
        nc.sync.dma_start(dst[:, c], r[:, c])


def _dma_head_rest(nc, dst, src_2d, head):
    """DMA a [E, E] weight into SBUF [P, EC, E] as two transfers: columns
    [0, head) first (what the first projection groups read), then the rest.
    Gets the attention pipeline started ~40us earlier than waiting for the
    full tensor."""
    r = src_2d.rearrange("(c p) o -> p c o", p=P)
    nc.sync.dma_start(dst[:, :, 0:head], r[:, :, 0:head])
    return lambda: nc.sync.dma_start(dst[:, :, head:], r[:, :, head:])


def _build():
    nc = bacc.Bacc("TRN2", target_bir_lowering=False, debug=False, num_devices=NCORES)

    xT = nc.dram_tensor("xT", [E, S], BF16, kind="ExternalInput").ap()
    wqT = nc.dram_tensor("wqT", [E, E], BF16, kind="ExternalInput").ap()
    wkT = nc.dram_tensor("wkT", [E, E], BF16, kind="ExternalInput").ap()
    wvT = nc.dram_tensor("wvT", [E, E], BF16, kind="ExternalInput").ap()
    woT = nc.dram_tensor("woT", [E, E], BF16, kind="ExternalInput").ap()
    b_o = nc.dram_tensor("b_o", [1, E], F32, kind="ExternalInput").ap()
    out = nc.dram_tensor("out", [SQ, E], F32, kind="ExternalOutput").ap()

    with tile.TileContext(nc) as tc:
        with (
            tc.tile_pool(name="persist", bufs=1) as persist,
            tc.tile_pool(name="ld", bufs=1) as ld,
            tc.tile_pool(name="probs_sb", bufs=4) as ppool,
            tc.tile_pool(name="norm_sb", bufs=2) as apool,
            tc.tile_pool(name="ysb", bufs=3) as ypool,
            tc.tile_pool(name="ps1", bufs=2, space="PSUM") as ps1,
            tc.tile_pool(name="psS", bufs=2, space="PSUM") as psS_pool,
            tc.tile_pool(name="psO", bufs=2, space="PSUM") as psO_pool,
        ):
            QT = persist.tile([P, EC, SQ], BF16)       # Q^T  (o on partitions)
            KT = persist.tile([P, EC, S], BF16)        # K^T
            # slab padded 65->80: dual-fp8 ldweights needs even, 16B-aligned
            # outer free-AP steps (s3_lw_dual_fp8_restrictions)
            VA8 = persist.tile([P, KTP, H, 2, 80], FP8)  # V fp8 + ones col
            HT = persist.tile([P, EC, SQ], BF16)       # Hout^T
            bias_bc = persist.tile([P, E], F32)

            # V is host-scaled by 16 (see make_in_maps) so fp8 e4m3 values
            # avoid the denormal range; the ones column is 16 so the
            # numerator/denominator ratio is exactly Hout.
            nc.vector.memset(VA8[:, :, :, :, D:D + 1], 16.0)

            xTs = ld.tile([P, EC, S], BF16)
            wq_s = ld.tile([P, EC, E], BF16)
            wq_rest = _dma_head_rest(nc, wq_s, wqT, P)

            def qproj_group(c, qb):
                """Q^T for output chunk c, q-block qb (8 MMs + 1 cast)."""
                ps = ps1.tile([P, QB], F32, tag="ps", name="psq")
                for ec in range(EC):
                    nc.tensor.matmul(
                        ps[:],
                        wq_s[:, ec, c * P:(c + 1) * P],
                        xTs[:, ec, qb * QB:(qb + 1) * QB],
                        start=(ec == 0), stop=(ec == EC - 1),
                    )
                nc.vector.tensor_copy(QT[:, c, qb * QB:(qb + 1) * QB], ps[:])

            def kproj_group(wk_s, c, kb):
                ps = ps1.tile([P, QB], F32, tag="ps", name="psk")
                for ec in range(EC):
                    nc.tensor.matmul(
                        ps[:],
                        wk_s[:, ec, c * P:(c + 1) * P],
                        xTs[:, ec, kb * QB:(kb + 1) * QB],
                        start=(ec == 0), stop=(ec == EC - 1),
                    )
                nc.vector.tensor_copy(KT[:, c, kb * QB:(kb + 1) * QB], ps[:])

            def vproj_group(wv_s, tt, ob):
                """V for token tile tt, heads [8*ob, 8*ob+8) -> VA8 (1 copy)."""
                ps = ps1.tile([P, 8, D], F32, tag="ps", name="psv")
                for ec in range(EC):
                    nc.tensor.matmul(
                        ps[:],
                        xTs[:, ec, tt * P:(tt + 1) * P],
                        wv_s[:, ec, ob * QB:(ob + 1) * QB],
                        start=(ec == 0), stop=(ec == EC - 1),
                    )
                nc.vector.tensor_copy(
                    VA8[:, tt // 2, ob * 8:(ob + 1) * 8, tt % 2, 0:D], ps[:]
                )

            def outproj_group(wo_s, tt, ub):
                ps = ps1.tile([P, QB], F32, tag="ps", name="psy")
                for oc in range(EC):
                    nc.tensor.matmul(
                        ps[:],
                        HT[:, oc, tt * P:(tt + 1) * P],
                        wo_s[:, oc, ub * QB:(ub + 1) * QB],
                        start=(oc == 0), stop=(oc == EC - 1),
                    )
                y = ypool.tile([P, QB], F32, tag="y", name="y")
                nc.vector.tensor_add(y[:], ps[:], bias_bc[:, ub * QB:(ub + 1) * QB])
                nc.sync.dma_start(
                    out[tt * P:(tt + 1) * P, ub * QB:(ub + 1) * QB], y[:]
                )

            def attn_headpair(hp, qb, fillers=None):
                """Attention for head pair hp (heads 2hp, 2hp+1), q-block qb.

                fillers: optional {slot: [closures]} of dense PE work emitted
                at the top of k-tile slot 2*kg -- keeps the PE from idling
                (HAM clock-gate) while ACT runs exp.

                Per k-tile, both heads' QK MMs write one psS tile [P,2,QB]
                (hi on dim 1): disjoint 64-row PE row groups (tile_position
                auto-derives (0,0)/(64,0)) with adjacent issue -> the array
                runs the pair CONCURRENTLY, halving the QK slot time.  One
                exp instruction then covers both heads (same N=1024).
                """
                q0 = qb * QB
                psO = [
                    psO_pool.tile([D + 1, QB], F32, tag="psO", name=f"psO{hi}")
                    for hi in range(2)
                ]
                n_groups = KT_TILES // KG

                def emit_pv(kg, probs):
                    # one DoubleRow MM per head covers both k-tiles of kg
                    for hi in range(2):
                        h = hp * 2 + hi
                        nc.tensor.matmul(
                            psO[hi][:],
                            VA8[:, kg, h, :, 0:D + 1],
                            probs[:, :, hi, :],
                            start=(kg == 0), stop=(kg == n_groups - 1),
                            perf_mode=DR,
                        )

                prev = None  # (kg, probs): PV is pipelined one group behind
                for kg in range(n_groups):
                    # probs [P, kt-in-group, hi, QB]: PV's DoubleRow rhs for
                    # head hi is the 3D view [:, :, hi, :] (k-pair on dim 1)
                    probs = ppool.tile(
                        [P, KG, 2, QB], FP8, tag="probs", name="probs"
                    )
                    for kt2 in range(KG):
                        kt = kg * KG + kt2
                        psS = psS_pool.tile([P, 2, QB], F32, tag="psS", name="psS")
                        for hi in range(2):
                            r0 = hi * D
                            nc.tensor.matmul(
                                psS[:, hi, :],
                                KT[r0:r0 + D, hp, kt * P:(kt + 1) * P],
                                QT[r0:r0 + D, hp, q0:q0 + QB],
                                start=True, stop=True,
                            )
                        nc.scalar.activation(
                            probs[:, kt2], psS[:], EXP, scale=0.125
                        )
                    # PV consumes the PREVIOUS group's probs so the PE never
                    # waits on ACT (head-of-line stall -> HAM clock drop)
                    if prev is not None:
                        emit_pv(*prev)
                    prev = (kg, probs)
                    # fillers LAST within the slot: QK(kg) must reach the PE
                    # first so exp(kg) starts the moment exp(kg-1) retires --
                    # the ACT stream is the pacer, fillers absorb the slack
                    for f in (fillers or {}).get(2 * kg, []):
                        f()
                emit_pv(*prev)
                # normalize: Hout^T = O^T * (1/denom), denom = psO row D
                for hi in range(2):
                    # custom-DVE ops require base partition 0: copy denom row out
                    dn = apool.tile([1, QB], F32, tag="dn", name="dn")
                    nc.vector.tensor_copy(dn[:], psO[hi][D:D + 1, :])
                    recip = apool.tile([1, QB], F32, tag="recip", name="recip")
                    nc.vector.reciprocal_approx_fast(recip[:], dn[:])
                    rb_sb = apool.tile([D, QB], F32, tag="rbsb", name="rbsb")
                    nc.gpsimd.partition_broadcast(rb_sb[:], recip[:])
                    nc.vector.tensor_mul(
                        HT[hi * D:(hi + 1) * D, hp, q0:q0 + QB],
                        psO[hi][0:D, :],
                        rb_sb[:],
                    )

            with tc.tile_pool(name="ld_kv", bufs=1) as ld_kv:
                wk_s = ld_kv.tile([P, EC, E], BF16)
                wk_rest = _dma_head_rest(nc, wk_s, wkT, P)
                # x after the small wq/wk heads, in S-quarters so the first
                # projection groups (cols 0-511) unblock after 1MB; the wv
                # head (1MB, needed by vproj from ~kg1) goes between x
                # quarters, the weight rests last
                r_xT = xT.rearrange("(c p) t -> p c t", p=P)

                def x_quarter(q):
                    w = S // 4
                    for c in range(EC):
                        nc.sync.dma_start(
                            xTs[:, c, q * w:(q + 1) * w],
                            r_xT[:, c, q * w:(q + 1) * w],
                        )

                x_quarter(0)
                x_quarter(1)
                wv_s = ld_kv.tile([P, EC, E], BF16)
                wv_rest = _dma_head_rest(nc, wv_s, wvT, QB)
                x_quarter(2)
                x_quarter(3)
                # rests queue behind all three heads: the pipeline-start
                # chain (qproj c0 / kproj c0 / vproj ob0) unblocks first
                wq_rest()
                wk_rest()
                wv_rest()

                # --------- phase A (minimal upfront) + B: attention(qb0) ------
                # only what QK(hp0,kg0-1) + pipelined PV(kg0) need; the rest
                # of the old phase A is JIT filler inside hp0's slots
                qproj_group(0, 0)
                for kb in range(4):
                    kproj_group(wk_s, 0, kb)
                vproj_group(wv_s, 0, 0)
                vproj_group(wv_s, 1, 0)

                def fB(hp):
                    # During attn(hp, qb0)  (slot key = k-tile index 0..15):
                    #  hp0: vproj(ob0) tt2-15 JIT two slots ahead of its own
                    #       PV, K/Q chunk-1 in the late slots
                    #  hp>=2: kt0: kproj(hp, kb3)  (read by this hp at kt>=12)
                    #  hp>=1: kt2,4,6: kproj(hp+1, kb0..2), kt8: qproj(hp+1);
                    #       late slots: V heads 8-15 (must land before hp4's
                    #       PV reads k-tile tt), then Q^T(qb1) chunk 0.
                    if hp == 0:
                        d = {}
                        for s in range(7):
                            d[2 * s] = [
                                lambda tt=2 + 2 * s: vproj_group(wv_s, tt, 0),
                                lambda tt=3 + 2 * s: vproj_group(wv_s, tt, 0),
                            ]
                        for s, kb in ((6, 0), (8, 1), (10, 2), (12, 3)):
                            d[s].append(
                                lambda kb=kb: kproj_group(wk_s, 1, kb))
                        d[14] = [lambda: qproj_group(1, 0)]
                        return d
                    d = {}
                    if hp >= 2:
                        d[0] = [lambda: kproj_group(wk_s, hp, 3)]
                    if hp < EC - 1:
                        for kb in range(3):
                            d[2 + 2 * kb] = [
                                lambda kb=kb: kproj_group(wk_s, hp + 1, kb)
                            ]
                        d[8] = [lambda: qproj_group(hp + 1, 0)]
                    vslots = {
                        1: [(10, 0), (12, 1), (14, 2), (10, 3), (12, 4),
                            (14, 5)],
                        2: [(10, 6), (12, 7), (14, 8)],
                        3: [(10, 9), (12, 10), (14, 11)],
                        4: [(6, 12), (8, 13), (10, 14), (12, 15)],
                        5: [(10, None)],  # qproj(0, qb1)
                    }
                    for sl, tt in vslots.get(hp, []):
                        ff = (
                            (lambda: qproj_group(0, 1)) if tt is None
                            else (lambda tt=tt: vproj_group(wv_s, tt, 1))
                        )
                        d.setdefault(sl, []).append(ff)
                    return d

                for hp in range(EC):
                    attn_headpair(hp, 0, fB(hp))

            # wo / bias scope reuses the space freed by wk/wv
            with tc.tile_pool(name="ld_c", bufs=1) as ld_c:
                wo_s = ld_c.tile([P, EC, E], BF16)
                _dma_chunked(nc, wo_s, woT)

                # bias broadcast: [1,E] -> [128,E] on GpSimd, off the PE path
                bo_s = ld_c.tile([1, E], F32)
                nc.sync.dma_start(bo_s[:], b_o)
                nc.gpsimd.partition_broadcast(bias_bc[:], bo_s[:])

                # ------------- phase C: attention(qb1) + filler -------------
                # filler: remaining Q^T(qb1) chunks + outproj of qb0 rows
                # (tt 0-3: every phase-B normalize is done by now; phase C is
                # ACT-bound with ~900ns/group of PE slack, so these are free)
                oslots = {
                    2: [(0, 0)], 3: [(0, 1), (1, 0)], 4: [(1, 1), (2, 0)],
                    5: [(2, 1), (3, 0)], 6: [(3, 1)],
                }

                def fC(hp):
                    d = {}
                    if hp < EC - 1:
                        d[2] = [lambda: qproj_group(hp + 1, 1)]
                    for tt, ub in oslots.get(hp, []):
                        d.setdefault(10, []).append(
                            lambda tt=tt, ub=ub: outproj_group(wo_s, tt, ub)
                        )
                    return d

                for hp in range(EC):
                    attn_headpair(hp, 1, fC(hp))

                # ------------- phase D: outproj of qb1 rows -----------------
                for tt in range(4, 8):
                    for ub in range(E // QB):
                        outproj_group(wo_s, tt, ub)

    nc.compile()
    return nc


def get_nc():
    if "nc" not in _CACHE:
        _CACHE["nc"] = _build()
    return _CACHE["nc"]


def make_in_maps(x, W_q, W_k, W_v, W_o, b_o):
    bf16 = ml_dtypes.bfloat16
    wqT = np.ascontiguousarray(W_q.T).astype(bf16)
    wkT = np.ascontiguousarray(W_k.T).astype(bf16)
    # x16: keep fp8 e4m3 V values out of the denormal range (power of two,
    # exact in bf16; compensated by the 16.0 ones-column in VA8)
    wvT = np.ascontiguousarray(W_v.T * 16.0).astype(bf16)
    woT = np.ascontiguousarray(W_o.T).astype(bf16)
    bo2 = np.ascontiguousarray(b_o.reshape(1, E)).astype(np.float32)

    in_maps = []
    for core in range(NCORES):
        b, half = core // 2, core % 2
        xb_T = np.ascontiguousarray(x[b].T)  # [E, S]
        if half == 1:
            # rotate so this core's queries are always columns [0, SQ)
            xb_T = np.concatenate([xb_T[:, SQ:], xb_T[:, :SQ]], axis=1)
        in_maps.append({
            "xT": np.ascontiguousarray(xb_T).astype(bf16),
            "wqT": wqT, "wkT": wkT, "wvT": wvT, "woT": woT,
            "b_o": bo2,
        })
    return in_maps


def run(x, W_q, W_k, W_v, W_o, b_o, **spmd_kwargs):
    nc = get_nc()
    in_maps = make_in_maps(x, W_q, W_k, W_v, W_o, b_o)
    res = run_bass_kernel_spmd(nc, in_maps, core_ids=list(range(NCORES)), **spmd_kwargs)
    out = np.empty((B, S, E), dtype=np.float32)
    for core in range(NCORES):
        b, half = core // 2, core % 2
        out[b, half * SQ:(half + 1) * SQ, :] = res.results[core]["out"]
    return out, res


def kernel(x, W_q, W_k, W_v, W_o, b_o):
    out, _ = run(x, W_q, W_k, W_v, W_o, b_o)
    return out



# revision 15
# speedup vs baseline: 1.0340x; 1.0340x over previous
"""Trainium2 Bass kernel: 16-head self-attention (B=4, S=2048, E=1024).

Reference math:
  Q = x @ W_q.T ; K = x @ W_k.T ; V = x @ W_v.T      (split into 16 heads of 64)
  A = softmax(Q K^T / sqrt(64)) ; Hout = A @ V
  out = concat_heads(Hout) @ W_o.T + b_o

Sharding: data-parallel over (batch, seq-half) -> 8 cores, no collectives.
Core i handles batch i//2 and query rows [ (i%2)*1024, (i%2+1)*1024 ).
K/V are computed for the full 2048-token sequence on every core.  Odd cores
receive x^T with the two sequence halves swapped so the "first 1024 columns"
are always the core's queries; softmax is permutation-invariant over keys.

Projections and QK^T are bf16 (fp8 projections fail the 2e-2 error budget:
the uniform +-1/32 weights quantize badly in e4m3; QK's contraction is only
64, where DoubleRow measured 2x SLOWER).  Only the PV matmul uses fp8
e4m3 via MatmulPerfMode.DoubleRow, which contracts TWO 128-partition
k-tiles per instruction at the bf16 streaming rate (216ns per [*,2,512]
MM = 2x bf16 FLOPs).  Measured end-to-end rel err 1.67e-2 (gate 2e-2),
matching a numpy simulation of the same quantization points exactly.

On-chip layout (feature-on-partition, "transposed"):
  xT  [e, t]  bf16 (host pre-transposed)
  WqT/WkT/WvT/WoT [e, o] bf16; WvT host-scaled x16 (keeps fp8 V out of
      e4m3 denormals; compensated by a 16.0 ones-column)
  Q^T [o, t_q], K^T [o, t_k] bf16
  V stored fp8, kt-paired, slab padded to 80B (dual-fp8 ldweights needs
      even, 16B-aligned outer free-AP steps):
     VA8[k, ktp, h, j, 0:64] = 16*V(token (2*ktp+j)*128+k, head h),
     VA8[..., 64] = 16
  S^T[k, q] = matmul(lhsT=K^T_head, rhs=Q^T_head)   bf16, contraction d=64
  P = exp(S^T / 8) -> fp8 e4m3 directly from the ACT engine (no max
      subtraction needed: scores are N(0, ~0.33^2); the denominator uses
      the same quantized P so softmax weights still sum to 1)
  O^T[d,q] + denom row = DoubleRow matmul(lhsT=VA8 pair [128,2,65], rhs=P)
  Hout^T = O^T * (1/denom)
  Y[t, u] = matmul(lhsT=Hout^T tile, rhsT=WoT bf16) + b_o

Scheduling (trace-derived model): the binding constraint is the ACT
engine's exp stream plus PSUM-fabric contention between ACT psum reads
and PE psum writes — each [128,2,512] k-group costs ~3.1us regardless of
how much PE work overlaps it (a pure QK stream runs 186ns/MM; the same
stream next to ACT exp runs ~512ns/MM).  So all projection/output
matmuls are interleaved into the attention loop as filler where they
execute essentially for free, and PV is software-pipelined one k-group
behind exp so the PE never head-of-line blocks on ACT:
  phase A: K^T(kb0-2), V(heads 0-7), Q^T(chunk0,qb0)   -- dense matmuls
  phase B: attention(qb0) + K/Q/V-remainder fillers
  phase C: attention(qb1) + Q^T(qb1) fillers
  phase D: output projection (all rows)
"""

import sys

for _p in ("/opt/trn_rl_repo",):
    if _p not in sys.path:
        sys.path.append(_p)

import numpy as np
import ml_dtypes

import concourse.bass as bass
import concourse.mybir as mybir
import concourse.tile as tile
from concourse import bacc
from concourse.bass_utils import run_bass_kernel_spmd

B, S, E = 4, 2048, 1024
H, D = 16, 64
P = 128
SQ = S // 2  # queries per core
NCORES = 8
EC = E // P  # 8 feature chunks
ECP = EC // 2  # 4 chunk pairs (DoubleRow)
KT_TILES = S // P  # 16 key tiles
KTP = KT_TILES // 2  # 8 key-tile pairs
QB = 512  # q block (matmul free dim / PSUM bank width)
KG = 2  # k-tiles per exp group (ACT instruction spans KG*512 psum cols)
NQB = SQ // QB  # 2 q-blocks per core

BF16 = mybir.dt.bfloat16
FP8 = mybir.dt.float8e4
F32 = mybir.dt.float32
EXP = mybir.ActivationFunctionType.Exp
DR = mybir.MatmulPerfMode.DoubleRow

_CACHE = {}


def _dma_chunked(nc, dst, src_2d):
    """DMA a [E, N] DRAM tensor into SBUF [P, EC, N], one chunk at a time so
    consumers of chunk 0 don't wait for the whole transfer."""
    r = src_2d.rearrange("(c p) t -> p c t", p=P)
    for c in range(EC):
        nc.sync.dma_start(dst[:, c], r[:, c])


def _dma_head_rest(nc, dst, src_2d, head):
    """DMA a [E, E] weight into SBUF [P, EC, E] as two transfers: columns
    [0, head) first (what the first projection groups read), then the rest.
    Gets the attention pipeline started ~40us earlier than waiting for the
    full tensor."""
    r = src_2d.rearrange("(c p) o -> p c o", p=P)
    nc.sync.dma_start(dst[:, :, 0:head], r[:, :, 0:head])
    return lambda: nc.sync.dma_start(dst[:, :, head:], r[:, :, head:])


def _build():
    nc = bacc.Bacc("TRN2", target_bir_lowering=False, debug=False, num_devices=NCORES)

    xT = nc.dram_tensor("xT", [E, S], BF16, kind="ExternalInput").ap()
    wqT = nc.dram_tensor("wqT", [E, E], BF16, kind="ExternalInput").ap()
    wkT = nc.dram_tensor("wkT", [E, E], BF16, kind="ExternalInput").ap()
    wvT = nc.dram_tensor("wvT", [E, E], BF16, kind="ExternalInput").ap()
    woT = nc.dram_tensor("woT", [E, E], BF16, kind="ExternalInput").ap()
    b_o = nc.dram_tensor("b_o", [1, E], BF16, kind="ExternalInput").ap()
    out = nc.dram_tensor("out", [SQ, E], F32, kind="ExternalOutput").ap()

    with tile.TileContext(nc) as tc:
        with (
            tc.tile_pool(name="persist", bufs=1) as persist,
            tc.tile_pool(name="ld", bufs=1) as ld,
            tc.tile_pool(name="probs_sb", bufs=3) as ppool,
            tc.tile_pool(name="norm_sb", bufs=2) as apool,
            tc.tile_pool(name="ysb", bufs=2) as ypool,
            tc.tile_pool(name="ps1", bufs=2, space="PSUM") as ps1,
            tc.tile_pool(name="psS", bufs=2, space="PSUM") as psS_pool,
            tc.tile_pool(name="psO", bufs=2, space="PSUM") as psO_pool,
        ):
            QT = persist.tile([P, EC, SQ], BF16)       # Q^T  (o on partitions)
            KT = persist.tile([P, EC, S], BF16)        # K^T
            # slab padded 65->80: dual-fp8 ldweights needs even, 16B-aligned
            # outer free-AP steps (s3_lw_dual_fp8_restrictions)
            VA8 = persist.tile([P, KTP, H, 2, 80], FP8)  # V fp8 + ones col
            HT = persist.tile([P, EC, SQ], BF16)       # Hout^T
            bias_bc = persist.tile([P, E], BF16)
            y1 = persist.tile([P, SQ // P, 2, QB], BF16)  # outproj half-sums

            # V is host-scaled by 16 (see make_in_maps) so fp8 e4m3 values
            # avoid the denormal range; the ones column is 16 so the
            # numerator/denominator ratio is exactly Hout.
            nc.vector.memset(VA8[:, :, :, :, D:D + 1], 16.0)

            xTs = ld.tile([P, EC, S], BF16)
            wq_s = ld.tile([P, EC, E], BF16)
            wq_rest = _dma_head_rest(nc, wq_s, wqT, P)

            def qproj_group(c, qb):
                """Q^T for output chunk c, q-block qb (8 MMs + 1 cast)."""
                ps = ps1.tile([P, QB], F32, tag="ps", name="psq")
                for ec in range(EC):
                    nc.tensor.matmul(
                        ps[:],
                        wq_s[:, ec, c * P:(c + 1) * P],
                        xTs[:, ec, qb * QB:(qb + 1) * QB],
                        start=(ec == 0), stop=(ec == EC - 1),
                    )
                nc.vector.tensor_copy(QT[:, c, qb * QB:(qb + 1) * QB], ps[:])

            def kproj_group(wk_s, c, kb):
                ps = ps1.tile([P, QB], F32, tag="ps", name="psk")
                for ec in range(EC):
                    nc.tensor.matmul(
                        ps[:],
                        wk_s[:, ec, c * P:(c + 1) * P],
                        xTs[:, ec, kb * QB:(kb + 1) * QB],
                        start=(ec == 0), stop=(ec == EC - 1),
                    )
                nc.vector.tensor_copy(KT[:, c, kb * QB:(kb + 1) * QB], ps[:])

            def vproj_group(wv_s, tt, ob):
                """V for token tile tt, heads [8*ob, 8*ob+8) -> VA8 (1 copy)."""
                ps = ps1.tile([P, 8, D], F32, tag="ps", name="psv")
                for ec in range(EC):
                    nc.tensor.matmul(
                        ps[:],
                        xTs[:, ec, tt * P:(tt + 1) * P],
                        wv_s[:, ec, ob * QB:(ob + 1) * QB],
                        start=(ec == 0), stop=(ec == EC - 1),
                    )
                nc.vector.tensor_copy(
                    VA8[:, tt // 2, ob * 8:(ob + 1) * 8, tt % 2, 0:D], ps[:]
                )

            def outproj_part1(wo_s, tt, ub):
                """Half-contraction (head-pairs 0-3) of the output projection
                for token tile tt / output block ub.  Eligible as soon as
                hp0-3 are normalized -> fills the PE slack of hp4-6 instead
                of serializing behind hp7.  Result parked in SBUF bf16
                (+bias; bf16 rounding of the half-sum is ~0.2% of |Y|)."""
                ps = ps1.tile([P, QB], F32, tag="ps", name="psy")
                for oc in range(4):
                    nc.tensor.matmul(
                        ps[:],
                        HT[:, oc, tt * P:(tt + 1) * P],
                        wo_s[:, oc, ub * QB:(ub + 1) * QB],
                        start=(oc == 0), stop=(oc == 3),
                    )
                nc.vector.tensor_add(
                    y1[:, tt, ub], ps[:], bias_bc[:, ub * QB:(ub + 1) * QB]
                )

            def outproj_part2(wo_s, tt, ub):
                ps = ps1.tile([P, QB], F32, tag="ps", name="psy")
                for oc in range(4, EC):
                    nc.tensor.matmul(
                        ps[:],
                        HT[:, oc, tt * P:(tt + 1) * P],
                        wo_s[:, oc, ub * QB:(ub + 1) * QB],
                        start=(oc == 4), stop=(oc == EC - 1),
                    )
                y = ypool.tile([P, QB], F32, tag="y", name="y")
                nc.vector.tensor_add(y[:], ps[:], y1[:, tt, ub])
                nc.sync.dma_start(
                    out[tt * P:(tt + 1) * P, ub * QB:(ub + 1) * QB], y[:]
                )

            def attn_headpair(hp, qb, fillers=None):
                """Attention for head pair hp (heads 2hp, 2hp+1), q-block qb.

                fillers: optional {slot: [closures]} of dense PE work emitted
                at the top of k-tile slot 2*kg -- keeps the PE from idling
                (HAM clock-gate) while ACT runs exp.

                Per k-tile, both heads' QK MMs write one psS tile [P,2,QB]
                (hi on dim 1): disjoint 64-row PE row groups (tile_position
                auto-derives (0,0)/(64,0)) with adjacent issue -> the array
                runs the pair CONCURRENTLY, halving the QK slot time.  One
                exp instruction then covers both heads (same N=1024).
                """
                q0 = qb * QB
                psO = [
                    psO_pool.tile([D + 1, QB], F32, tag="psO", name=f"psO{hi}")
                    for hi in range(2)
                ]
                n_groups = KT_TILES // KG

                def emit_pv(kg, probs):
                    # one DoubleRow MM per head covers both k-tiles of kg
                    for hi in range(2):
                        h = hp * 2 + hi
                        nc.tensor.matmul(
                            psO[hi][:],
                            VA8[:, kg, h, :, 0:D + 1],
                            probs[:, :, hi, :],
                            start=(kg == 0), stop=(kg == n_groups - 1),
                            perf_mode=DR,
                        )

                prev = None  # (kg, probs): PV is pipelined one group behind
                for kg in range(n_groups):
                    # probs [P, kt-in-group, hi, QB]: PV's DoubleRow rhs for
                    # head hi is the 3D view [:, :, hi, :] (k-pair on dim 1)
                    probs = ppool.tile(
                        [P, KG, 2, QB], FP8, tag="probs", name="probs"
                    )
                    for kt2 in range(KG):
                        kt = kg * KG + kt2
                        psS = psS_pool.tile([P, 2, QB], F32, tag="psS", name="psS")
                        for hi in range(2):
                            r0 = hi * D
                            nc.tensor.matmul(
                                psS[:, hi, :],
                                KT[r0:r0 + D, hp, kt * P:(kt + 1) * P],
                                QT[r0:r0 + D, hp, q0:q0 + QB],
                                start=True, stop=True,
                            )
                        nc.scalar.activation(
                            probs[:, kt2], psS[:], EXP, scale=0.125
                        )
                    # PV consumes the PREVIOUS group's probs so the PE never
                    # waits on ACT (head-of-line stall -> HAM clock drop)
                    if prev is not None:
                        emit_pv(*prev)
                    prev = (kg, probs)
                    # fillers LAST within the slot: QK(kg) must reach the PE
                    # first so exp(kg) starts the moment exp(kg-1) retires --
                    # the ACT stream is the pacer, fillers absorb the slack
                    for f in (fillers or {}).get(2 * kg, []):
                        f()
                emit_pv(*prev)
                # normalize: Hout^T = O^T * (1/denom), denom = psO row D
                for hi in range(2):
                    # custom-DVE ops require base partition 0: copy denom row out
                    dn = apool.tile([1, QB], F32, tag="nrm", name="dn")
                    nc.vector.tensor_copy(dn[:], psO[hi][D:D + 1, :])
                    recip = apool.tile([1, QB], F32, tag="nrm", name="recip")
                    nc.vector.reciprocal_approx_fast(recip[:], dn[:])
                    rb_sb = apool.tile([D, QB], F32, tag="rbsb", name="rbsb")
                    nc.gpsimd.partition_broadcast(rb_sb[:], recip[:])
                    nc.vector.tensor_mul(
                        HT[hi * D:(hi + 1) * D, hp, q0:q0 + QB],
                        psO[hi][0:D, :],
                        rb_sb[:],
                    )

            # Pass order is hp-major, qb-minor: attn(0,0) attn(0,1) attn(1,0)
            # ... so each pass's exp stream (16 x 1147ns = the ACT floor)
            # paces the kernel and ALL projection/outproj work rides as PE
            # filler in the ~10us/pass of PE slack.  Filler placement:
            #   (0,0): vproj ob0 JIT (+qproj(0,qb1))     (0,1): K/Q chunk 1
            #   (1..3,0): vproj ob1                      (c,1): K/Q chunk c+1
            #   (4..6,0): outproj part1 (hp0-3 halves)   (7,1): part2 of qb0
            #   tail: part2 of qb1
            with tc.tile_pool(name="ld_k", bufs=1) as ld_k:
                wk_s = ld_k.tile([P, EC, E], BF16)
                wk_rest = _dma_head_rest(nc, wk_s, wkT, P)
                # x after the small wq/wk heads, in S-quarters so the first
                # projection groups (cols 0-511) unblock after 1MB; the wv
                # head (1MB, needed by vproj from ~kg1) goes between x
                # quarters, the weight rests last
                r_xT = xT.rearrange("(c p) t -> p c t", p=P)

                def x_quarter(q):
                    w = S // 4
                    for c in range(EC):
                        nc.sync.dma_start(
                            xTs[:, c, q * w:(q + 1) * w],
                            r_xT[:, c, q * w:(q + 1) * w],
                        )

                with tc.tile_pool(name="ld_v", bufs=1) as ld_v:
                    wv_s = ld_v.tile([P, EC, E], BF16)
                    x_quarter(0)
                    x_quarter(1)
                    wv_rest = _dma_head_rest(nc, wv_s, wvT, QB)
                    x_quarter(2)
                    x_quarter(3)
                    # rests queue behind all three heads: the pipeline-start
                    # chain (qproj c0 / kproj c0 / vproj ob0) unblocks first
                    wq_rest()
                    wk_rest()
                    wv_rest()

                    # --------- minimal upfront: what pass (0,0) needs ------
                    qproj_group(0, 0)
                    for kb in range(4):
                        kproj_group(wk_s, 0, kb)
                    vproj_group(wv_s, 0, 0)
                    vproj_group(wv_s, 1, 0)

                    def kq_next(hp):
                        # K + Q chunks for head-pair hp+1, in hp's qb1 pass
                        d = {}
                        if hp < EC - 1:
                            for kb in range(4):
                                d[2 * kb] = [
                                    lambda kb=kb: kproj_group(wk_s, hp + 1, kb)
                                ]
                            d[8] = [lambda: qproj_group(hp + 1, 0)]
                            d[10] = [lambda: qproj_group(hp + 1, 1)]
                        return d

                    def f_v(hp, qb):
                        if qb == 1:
                            return kq_next(hp)
                        d = {}
                        if hp == 0:
                            # vproj(ob0) tt2-15 JIT two slots ahead of its
                            # own PV; Q^T(0,qb1) for the very next pass
                            for s in range(7):
                                d[2 * s] = [
                                    lambda tt=2 + 2 * s: vproj_group(wv_s, tt, 0),
                                    lambda tt=3 + 2 * s: vproj_group(wv_s, tt, 0),
                                ]
                            d[14] = [lambda: qproj_group(0, 1)]
                            return d
                        # hp1-3 qb0: V heads 8-15 (needed from hp4 pass 0)
                        vt = {1: range(0, 6), 2: range(6, 12), 3: range(12, 16)}
                        for i, tt in enumerate(vt[hp]):
                            d.setdefault(4 + 2 * i, []).append(
                                lambda tt=tt: vproj_group(wv_s, tt, 1)
                            )
                        return d

                    for hp in range(4):
                        attn_headpair(hp, 0, f_v(hp, 0))
                        attn_headpair(hp, 1, f_v(hp, 1))

                # wo / bias scope reuses the space freed by wv
                with tc.tile_pool(name="ld_o", bufs=1) as ld_o:
                    wo_s = ld_o.tile([P, EC, E], BF16)
                    _dma_chunked(nc, wo_s, woT)

                    # bias bcast: [1,E] -> [128,E] on GpSimd, off the PE path
                    bo_s = ld_o.tile([1, E], BF16)
                    nc.sync.dma_start(bo_s[:], b_o)
                    nc.gpsimd.partition_broadcast(bias_bc[:], bo_s[:])

                    # outproj part1 halves (any tt: hp0-3 normalized for both
                    # q-blocks by now), spread over the hp4-6 qb0 passes
                    p1 = {
                        4: [(0, 0), (0, 1), (1, 0), (1, 1), (2, 0)],
                        5: [(2, 1), (3, 0), (3, 1), (4, 0), (4, 1), (5, 0)],
                        6: [(5, 1), (6, 0), (6, 1), (7, 0), (7, 1)],
                    }

                    def f_o(hp, qb):
                        if qb == 1 and hp < EC - 1:
                            return kq_next(hp)
                        d = {}
                        if qb == 0 and hp in p1:
                            for i, (tt, ub) in enumerate(p1[hp]):
                                d.setdefault(4 + 2 * i, []).append(
                                    lambda tt=tt, ub=ub: outproj_part1(wo_s, tt, ub)
                                )
                        elif hp == EC - 1:
                            # part2 of qb0 rows: hp7's qb0 normalize lands
                            # ~2us into this pass
                            for i in range(8):
                                tt, ub = i // 2, i % 2
                                d.setdefault(2 * i, []).append(
                                    lambda tt=tt, ub=ub: outproj_part2(wo_s, tt, ub)
                                )
                        return d

                    for hp in range(4, EC):
                        attn_headpair(hp, 0, f_o(hp, 0))
                        attn_headpair(hp, 1, f_o(hp, 1))

                    # tail: part2 of the qb1 rows (needs hp7 qb1 normalize)
                    for tt in range(4, 8):
                        for ub in range(E // QB):
                            outproj_part2(wo_s, tt, ub)

    nc.compile()
    return nc


def get_nc():
    if "nc" not in _CACHE:
        _CACHE["nc"] = _build()
    return _CACHE["nc"]


def make_in_maps(x, W_q, W_k, W_v, W_o, b_o):
    bf16 = ml_dtypes.bfloat16
    wqT = np.ascontiguousarray(W_q.T).astype(bf16)
    wkT = np.ascontiguousarray(W_k.T).astype(bf16)
    # x16: keep fp8 e4m3 V values out of the denormal range (power of two,
    # exact in bf16; compensated by the 16.0 ones-column in VA8)
    wvT = np.ascontiguousarray(W_v.T * 16.0).astype(bf16)
    woT = np.ascontiguousarray(W_o.T).astype(bf16)
    bo2 = np.ascontiguousarray(b_o.reshape(1, E)).astype(bf16)

    in_maps = []
    for core in range(NCORES):
        b, half = core // 2, core % 2
        xb_T = np.ascontiguousarray(x[b].T)  # [E, S]
        if half == 1:
            # rotate so this core's queries are always columns [0, SQ)
            xb_T = np.concatenate([xb_T[:, SQ:], xb_T[:, :SQ]], axis=1)
        in_maps.append({
            "xT": np.ascontiguousarray(xb_T).astype(bf16),
            "wqT": wqT, "wkT": wkT, "wvT": wvT, "woT": woT,
            "b_o": bo2,
        })
    return in_maps


def run(x, W_q, W_k, W_v, W_o, b_o, **spmd_kwargs):
    nc = get_nc()
    in_maps = make_in_maps(x, W_q, W_k, W_v, W_o, b_o)
    res = run_bass_kernel_spmd(nc, in_maps, core_ids=list(range(NCORES)), **spmd_kwargs)
    out = np.empty((B, S, E), dtype=np.float32)
    for core in range(NCORES):
        b, half = core // 2, core % 2
        out[b, half * SQ:(half + 1) * SQ, :] = res.results[core]["out"]
    return out, res


def kernel(x, W_q, W_k, W_v, W_o, b_o):
    out, _ = run(x, W_q, W_k, W_v, W_o, b_o)
    return out

